# revision 75
# baseline (speedup 1.0000x reference)
"""Causal multi-head attention (fused QKV) on 8 Trainium2 NeuronCores.

Problem: x[2, 2048, 1024] @ W_qkv[1024, 3072] -> causal MHA, 16 heads,
head_dim 64 -> out [2, 2048, 1024].

Sharding: batch (2) x head-groups (4) = 8 shards; core c handles batch
c//4, heads 4*(c%4) .. 4*(c%4)+3.  Each core is fully independent (no
collectives).

Per-core layouts (host prepares, all matmul operands bf16):
  xT   [128, 16384]  x[b].T re-tiled [p, sc(4), dc(8), 512] (contraction
                     dim D split into 8 chunks of 128 partitions)
  w    [128, 6144]   W columns reordered + grouped for contiguous DMA:
                     [Q01 | K01 | V | Q23 | K23], each group dc-major
  qkb  [128, 4]      QK bias per 128-row chunk (column fc)
  vb   [128, 256]    V bias replicated across partitions
  outT [256, S]      output transposed: row 64*h+j, col s = out[b,s,h,j]

On-core algorithm:
  QK.T = w_qk.T @ x.T   -> [512, S] rows (Q_h0|Q_h1 | Q_h2|Q_h3 | K...)
  V    = x @ w_v        -> [S, 256], stored as [V_h|1] blocks of 65 cols
  per head pair pr, per q-chunk qc (512), per k-block kb (128, causal):
    S_T[k,q] = K@Q.T via 2 row-tiled (K=64) matmuls    (PE)
    P = exp(S_T/8)  PSUM -> SBUF bf16                  (ACT)
    diagonal kb: P *= upper-tri mask                   (DVE)
    av[hd+1, q] += [V|1].T @ P  (65th row = denom)     (PE)
  normalize: r = 1/denom on [1,512], partition-broadcast, multiply
  straight out of PSUM -> SBUF -> DMA out.

Schedule: the exp stream (ACT engine) paces attention; all projection
matmuls are interleaved between attention steps as PE filler so the
tensor engine never idles (keeps the PE p-state at full clock).  Blocks
alternate head pairs: b(0,0) b(1,0) b(0,1) b(1,1) ... with each block
carrying the QKT projection tiles needed two blocks later plus the V
tiles needed by its own new k-blocks.
"""

import sys

if "/opt/trn_rl_repo" not in sys.path:
    sys.path.insert(0, "/opt/trn_rl_repo")

import numpy as np
import ml_dtypes

import concourse.bass as bass
import concourse.mybir as mybir
import concourse.tile as tile
from concourse import bacc
from concourse.bass_utils import run_bass_kernel_spmd
from concourse.masks import make_upper_triangular

F32 = mybir.dt.float32
BF16 = mybir.dt.bfloat16
EXP = mybir.ActivationFunctionType.Exp
MULT = mybir.AluOpType.mult
ADD = mybir.AluOpType.add

N_CORES = 8
B, S, D = 2, 2048, 1024
N_HEAD = 16
HD = 64  # head dim
HPC = 4  # heads per core
FQK = 2 * HPC * HD  # 512 rows of QK.T
FV = HPC * HD  # 256 cols of V
NQC = S // 512  # 512-wide q chunks
NKB = S // 128  # 128-wide k blocks
NDC = D // 128  # 128-deep contraction chunks
VW = HD + 1  # V block width incl. ones column

# w_sb column offsets per group (each group is dc-major, bf16)
W_OFF = {0: 0, 2: NDC * 128, "v": 2 * NDC * 128, 1: 2 * NDC * 128 + NDC * 256,
         3: 3 * NDC * 128 + NDC * 256}
W_COLS = 4 * NDC * 128 + NDC * 256  # 6144


def x_off(sc, dc):
    return sc * (NDC * 512) + dc * 512


def build_mha_core(trace_sim=False):
    """Build the per-core Bass program."""
    nc = bacc.Bacc("TRN2", target_bir_lowering=False, debug=False)
    xT_d = nc.dram_tensor("xT", [128, NQC * NDC * 512], BF16, kind="ExternalInput")
    w_d = nc.dram_tensor("w", [128, W_COLS], BF16, kind="ExternalInput")
    qkb_d = nc.dram_tensor("qkb", [128, 4], F32, kind="ExternalInput")
    vb_d = nc.dram_tensor("vb", [128, FV], F32, kind="ExternalInput")
    outT_d = nc.dram_tensor("outT", [FV, S], F32, kind="ExternalOutput")

    with tile.TileContext(nc, trace_sim=trace_sim) as tc:
        with (
            tc.tile_pool(name="const", bufs=1) as const,
            tc.tile_pool(name="big", bufs=1) as big,
            tc.tile_pool(name="pp", bufs=4) as pp,
            tc.tile_pool(name="sm", bufs=3) as sm,
            tc.tile_pool(name="ps", bufs=2, space="PSUM") as ps,
        ):
            # PE warmup: dummy matmuls on a memset tile (no DMA deps) keep
            # the tensor engine busy from t~0 so its p-state ramps toward
            # full clock before the real projection arrives.
            warmsrc = const.tile([128, 512], BF16)
            nc.gpsimd.memset(warmsrc[:], 0.5)
            warm = ps.tile([128, 1024], F32, tag="sta", bufs=2, name="warm")
            for r in range(6):
                nc.tensor.matmul(
                    warm[:, 0:512],
                    warmsrc[:, 0:128],
                    warmsrc[:],
                    start=(r == 0),
                    stop=(r == 5),
                )

            mask = const.tile([128, 128], BF16)
            make_upper_triangular(nc, mask[:], val=1.0, diag=True)
            qkb = const.tile([128, 4], F32)
            vb = const.tile([128, FV], F32)

            w_sb = big.tile([128, W_COLS], BF16)
            xT_sb = big.tile([128, NQC * NDC * 512], BF16)
            qkt = big.tile([128, 4 * S], BF16)  # fc0..3 = Qh01,Qh23,Kh01,Kh23
            vcat = big.tile([128, NKB * HPC * VW], BF16)  # [V_h | 1] per kb,h

            # ---- input DMAs in consumption-priority order ----
            # Sync (SP) queue: what the first matmuls need; Scalar (ACT)
            # queue: the rest, issued in parallel to cut serialization.
            dma = nc.sync.dma_start
            dma2 = nc.scalar.dma_start
            g = lambda k: (W_OFF[k], W_OFF[k] + (NDC * 256 if k == "v" else NDC * 128))
            lo, hi = g(0)
            dma(out=w_sb[:, lo:hi], in_=w_d.ap()[:, lo:hi])  # Q heads 0,1
            for lo2, hi2 in ((0, 512), (512, 2048), (2048, 4096)):  # x sc0
                dma(out=xT_sb[:, lo2:hi2], in_=xT_d.ap()[:, lo2:hi2])
            lo, hi = g(2)
            dma(out=w_sb[:, lo:hi], in_=w_d.ap()[:, lo:hi])  # K heads 0,1
            lo, hi = g("v")
            dma(out=w_sb[:, lo:hi], in_=w_d.ap()[:, lo:hi])  # V
            lo = W_OFF[1]  # fc1 and fc3 are adjacent: one DMA
            dma(out=w_sb[:, lo:W_COLS], in_=w_d.ap()[:, lo:W_COLS])
            dma2(out=qkb[:], in_=qkb_d.ap())
            dma2(out=vb[:], in_=vb_d.ap())
            dma(out=xT_sb[:, 4096:8192], in_=xT_d.ap()[:, 4096:8192])  # x sc1
            dma(out=xT_sb[:, 8192:16384], in_=xT_d.ap()[:, 8192:16384])  # x sc2,3

            # ones column of vcat (denominator accumulator rows)
            nc.gpsimd.memset(
                vcat.rearrange("p (k h j) -> p k h j", k=NKB, h=HPC)[:, :, :, HD:VW],
                1.0,
            )

            # ---- projection tile emitters ----
            def emit_qkt(fc, sc, tag="pq"):
                pt = ps.tile([128, 512], F32, tag=tag, bufs=1, name=f"qk_{fc}_{sc}")
                for dc in range(NDC):
                    nc.tensor.matmul(
                        pt[:],
                        w_sb[:, W_OFF[fc] + dc * 128 : W_OFF[fc] + dc * 128 + 128],
                        xT_sb[:, x_off(sc, dc) : x_off(sc, dc) + 512],
                        start=(dc == 0),
                        stop=(dc == NDC - 1),
                    )
                nc.vector.tensor_scalar_add(
                    qkt[:, fc * S + sc * 512 : fc * S + sc * 512 + 512],
                    pt[:],
                    qkb[:, fc : fc + 1],
                )

            def emit_v(kc):
                pt = ps.tile([128, FV], F32, tag="pv", bufs=1, name=f"v_{kc}")
                sc, r = kc // 4, kc % 4
                for dc in range(NDC):
                    nc.tensor.matmul(
                        pt[:, 0:FV],
                        xT_sb[:, x_off(sc, dc) + r * 128 : x_off(sc, dc) + r * 128 + 128],
                        w_sb[:, W_OFF["v"] + dc * 256 : W_OFF["v"] + dc * 256 + 256],
                        start=(dc == 0),
                        stop=(dc == NDC - 1),
                    )
                nc.vector.tensor_tensor(
                    out=vcat.rearrange("p (k h j) -> p k h j", k=NKB, h=HPC)[
                        :, kc, :, 0:HD
                    ],
                    in0=pt[:, 0:FV].rearrange("p (h j) -> p h j", h=HPC),
                    in1=vb.rearrange("p (h j) -> p h j", h=HPC),
                    op=ADD,
                )

            # ---- attention block: head pair pr, q-chunk qc ----
            # fillers: list of (step, thunk) — projection tiles emitted
            # after the given attention step's matmuls (PE filler).
            def attn_block(pr, qc, fillers=(), last=False):
                qoff = pr * S
                koff = (2 + pr) * S
                nkb = 4 * qc + 4
                av = [
                    ps.tile([VW, 512], F32, tag="av", bufs=2, name=f"av_{pr}_{qc}_{i}")
                    for i in (0, 1)
                ]
                fill_pos = {}
                for step, f in fillers:
                    fill_pos.setdefault(max(0, min(step, nkb - 1)), []).append(f)

                for kb in range(nkb):
                    # fillers emit before this step's matmuls
                    for f in fill_pos.get(kb, ()):
                        f()
                    diag = kb // 4 == qc
                    off = 128 * (kb % 4) if diag else 0
                    st = ps.tile(
                        [128, 1024], F32, tag="sta", bufs=2, name=f"st_{pr}_{qc}_{kb}"
                    )
                    for i in (1, 0):
                        nc.tensor.matmul(
                            st[:, i * 512 + off : i * 512 + 512],
                            qkt[64 * i : 64 * i + 64,
                                koff + kb * 128 : koff + kb * 128 + 128],
                            qkt[64 * i : 64 * i + 64,
                                qoff + qc * 512 + off : qoff + qc * 512 + 512],
                            start=True,
                            stop=True,
                        )
                    p_t = pp.tile([128, 1024], BF16, tag="p", name=f"p_{pr}_{qc}_{kb}")
                    nc.scalar.activation(
                        p_t.rearrange("p (h q) -> p h q", h=2)[:, :, off:512],
                        st.rearrange("p (h q) -> p h q", h=2)[:, :, off:512],
                        EXP,
                        scale=0.125,
                    )
                    if diag:
                        for i in (0, 1):
                            sl = p_t[:, i * 512 + off : i * 512 + off + 128]
                            nc.vector.tensor_tensor(out=sl, in0=sl, in1=mask[:], op=MULT)
                    for i in (1, 0):
                        h = 2 * pr + i
                        nc.tensor.matmul(
                            av[i][:, off:512],
                            vcat[:, (kb * HPC + h) * VW : (kb * HPC + h) * VW + VW],
                            p_t[:, i * 512 + off : i * 512 + 512],
                            start=(kb == 0),
                            stop=(kb == nkb - 1),
                        )
                # normalize: copy av out (frees the PSUM bank for the next
                # block), then r = 1/denom (row 0), broadcast, multiply in
                # place
                for i in (0, 1):
                    h = 2 * pr + i
                    if last:
                        # tail latency path: no successor needs the av bank
                        # freed, so skip the staging copy and read PSUM
                        # directly
                        ou = sm.tile([HD, 512], F32, tag="ou", name=f"ol_{pr}_{qc}_{i}")
                        s_t = sm.tile([1, 512], F32, tag="s", name=f"s_{pr}_{qc}_{i}")
                        nc.vector.tensor_copy(out=s_t[:], in_=av[i][HD : HD + 1, :])
                        src = av[i][0:HD, :]
                    else:
                        ou = sm.tile([VW, 512], F32, tag="ou", name=f"ou_{pr}_{qc}_{i}")
                        nc.vector.tensor_copy(out=ou[:], in_=av[i][:])
                        s_t = sm.tile([1, 512], F32, tag="s", name=f"s_{pr}_{qc}_{i}")
                        nc.vector.tensor_copy(out=s_t[:], in_=ou[HD : HD + 1, :])
                        src = ou[0:HD, :]
                    r1 = sm.tile([1, 512], F32, tag="r1", name=f"r1_{pr}_{qc}_{i}")
                    nc.vector.reciprocal_approx_fast(r1[:], s_t[:])
                    rr = sm.tile([64, 512], F32, tag="rr", name=f"rr_{pr}_{qc}_{i}")
                    nc.gpsimd.partition_broadcast(rr[:], r1[:])
                    nc.vector.tensor_tensor(
                        out=ou[0:HD, :], in0=src, in1=rr[:], op=MULT
                    )
                    nc.sync.dma_start(
                        out=outT_d.ap()[64 * h : 64 * h + 64, qc * 512 : qc * 512 + 512],
                        in_=ou[0:HD, :],
                    )

            # ---- interleaved schedule ----
            # Block order alternates head pairs; each block carries as
            # "fillers" the projection tiles needed later: its own diagonal
            # K tile (needed only from step 4*qc), just-in-time V tiles
            # (V kc needed at step kc), and the Q/K tiles that gate the
            # next blocks.
            Q = lambda fc, sc: (lambda: emit_qkt(fc, sc))
            V = lambda kc: (lambda: emit_v(kc))
            emit_qkt(0, 0)
            emit_qkt(2, 0)
            attn_block(0, 0, [(0, V(0)), (0, V(1)), (1, V(2)), (1, V(3)),
                              (2, Q(1, 0)), (3, Q(3, 0))])
            attn_block(1, 0, [(0, Q(0, 1)), (2, Q(2, 1))])
            attn_block(0, 1, [(0, V(4)), (1, V(5)), (2, V(6)), (3, V(7)),
                              (4, Q(1, 1)), (6, Q(3, 1))])
            attn_block(1, 1, [(0, Q(0, 2)), (4, Q(2, 2))])
            attn_block(0, 2, [(4, V(8)), (5, V(9)), (6, V(10)), (7, V(11)),
                              (8, Q(1, 2)), (9, Q(3, 2))])
            attn_block(1, 2, [(0, Q(1, 3)), (5, Q(0, 3)), (9, Q(3, 3))])
            attn_block(1, 3, [(8, V(12)), (9, V(13)), (10, V(14)), (11, V(15)),
                              (6, Q(2, 3))])
            attn_block(0, 3, [], last=True)
    nc.compile()
    return nc


def shard_inputs(x, W_qkv, b_qkv):
    """Full inputs -> list of 8 per-core input maps."""
    in_maps = []
    for c in range(N_CORES):
        b = c // (N_CORES // B)
        gidx = c % (N_CORES // B)
        heads = range(HPC * gidx, HPC * gidx + HPC)
        qcols = [h * 192 + j for h in heads for j in range(64)]
        kcols = [h * 192 + 64 + j for h in heads for j in range(64)]
        vcols = [h * 192 + 128 + j for h in heads for j in range(64)]
        cols = qcols + kcols + vcols
        w_sh = np.ascontiguousarray(W_qkv[:, cols], dtype=np.float32)
        b_sh = np.ascontiguousarray(b_qkv[cols], dtype=np.float32)
        qkb = np.ascontiguousarray(b_sh[:FQK].reshape(4, 128).T, dtype=np.float32)
        vb = np.ascontiguousarray(
            np.broadcast_to(b_sh[FQK:], (128, FV)), dtype=np.float32
        )
        # w groups, each dc-major: [Q01 | K01 | V | Q23 | K23]
        w4 = w_sh.reshape(NDC, 128, 768)
        w_host = np.concatenate(
            [
                w4[:, :, 0:128].transpose(1, 0, 2).reshape(128, NDC * 128),
                w4[:, :, 256:384].transpose(1, 0, 2).reshape(128, NDC * 128),
                w4[:, :, 512:768].transpose(1, 0, 2).reshape(128, NDC * 256),
                w4[:, :, 128:256].transpose(1, 0, 2).reshape(128, NDC * 128),
                w4[:, :, 384:512].transpose(1, 0, 2).reshape(128, NDC * 128),
            ],
            axis=1,
        ).astype(ml_dtypes.bfloat16)
        # x: [D, S] -> [p, sc, dc, 512]
        xT = np.ascontiguousarray(x[b].T, dtype=np.float32)
        x_host = np.ascontiguousarray(
            xT.reshape(NDC, 128, NQC, 512).transpose(1, 2, 0, 3).reshape(128, -1)
        ).astype(ml_dtypes.bfloat16)
        in_maps.append({"xT": x_host, "w": w_host, "qkb": qkb, "vb": vb})
    return in_maps


def gather_outputs(results):
    """8 per-core outT [256, S] -> full [B, S, D_H]."""
    out = np.empty((B, S, N_HEAD * HD), dtype=np.float32)
    for c in range(N_CORES):
        b = c // (N_CORES // B)
        gidx = c % (N_CORES // B)
        out[b, :, FV * gidx : FV * (gidx + 1)] = results[c]["outT"].T
    return out


_NC_CACHE = {}


def _get_nc():
    if "nc" not in _NC_CACHE:
        _NC_CACHE["nc"] = build_mha_core()
    return _NC_CACHE["nc"]


def kernel(x, W_qkv, b_qkv, _trace=False, _trace_kwargs=None):
    x = np.asarray(x, dtype=np.float32)
    W_qkv = np.asarray(W_qkv, dtype=np.float32)
    b_qkv = np.asarray(b_qkv, dtype=np.float32)
    nc = _get_nc()
    in_maps = shard_inputs(x, W_qkv, b_qkv)
    res = run_bass_kernel_spmd(
        nc, in_maps, list(range(N_CORES)), trace=_trace, **(_trace_kwargs or {})
    )
    out = gather_outputs(res.results)
    if _trace:
        kernel.last_results = res
    return out
